# revision 56
# baseline (speedup 1.0000x reference)
"""Trainium2 Bass kernel for nn_BilateralLayer (guided filter, FFT-conv reference).

Fully SBUF-resident pipeline over 9 x-chunks of 128 columns. The 33x33
Gaussian (separable, zeroed center) is applied as band matmuls:
  R1y: data-stationary matmuls produce the y-filtered planes directly
       TRANSPOSED ([x, y] layout); PSUM packed 5x96 wide per bank so one
       scalar copy drains 5 matmuls.
  R1x: band-stationary matmuls (B128 + B32 halo) -> 25 moment planes.
  solve: batched 3x3 adjugate solve split across DVE + GpSimd: covT and
       cofactor products on GpSimd, At/det/adjugate/a/b on DVE with
       replicated 9-plane adjugate storage so a-compute runs as three
       wide (3456-elem) multiplies.
  R2x: ab-stationary matmuls emit the x-filtered a/b planes transposed
       back to [y, x]; BL/BR halo matmuls sliced to their 16 live output
       columns; staged through DRAM (tcxdr) for stage C.
  C:   y-band matmuls + combine q = sum_c mean_a*I + mean_b' (+0.5);
       combine split DVE (k<3) / GpSimd (k=3), final cast+bias on GpSimd.

Sharding: 8 cores = 2 batch x 4 row-bands of 256 rows (halo 2R).
Self-contained: hardcodes shapes; host-side prep in numpy.
"""
import sys

if "/opt/trn_rl_repo" not in sys.path:
    sys.path.insert(0, "/opt/trn_rl_repo")

import numpy as np
import ml_dtypes
from contextlib import ExitStack

import concourse.bass as bass
import concourse.tile as tile
from concourse import bacc, mybir
from concourse.bass_utils import run_bass_kernel_spmd

bf16 = ml_dtypes.bfloat16
F32 = mybir.dt.float32
BF16 = mybir.dt.bfloat16
F8 = mybir.dt.float8e4
OP = mybir.AluOpType
AF = mybir.ActivationFunctionType

R = 16
EPS = 0.01
B, H, W, C, K = 2, 1024, 1024, 3, 4
N_CORES = 8
ROWS = 256
EXT = ROWS + 4 * R    # 320
MID = ROWS + 2 * R    # 288
NPL = 25
NAB = 16

g1 = np.exp(-0.5 * (np.arange(-R, R + 1) ** 2) / (R / 4.0) ** 2).astype(np.float64)
_S1 = float(g1.sum())
_S2D = _S1 * _S1 - 1.0


def _band(nk, nm, shift):
    M = np.zeros((nk, nm), np.float64)
    for ki in range(nk):
        for mo in range(nm):
            d = ki - mo + shift
            if 0 <= d <= 2 * R:
                M[ki, mo] = g1[d]
    return M


BANDS = {
    "B96a": _band(128, 96, 0) / _S1,
    "B128": _band(128, 128, 0) * (_S1 / _S2D),
    "B32": _band(32, 128, 128) * (_S1 / _S2D),
    "BL": _band(128, 128, -112) / _S1,
    "BM": _band(128, 128, R) / _S1,
    "BR": _band(128, 128, 144) / _S1,
    "B96b": _band(128, 96, 0) * (_S1 / _S2D),
    "B96c2": _band(128, 64, -32) * (_S1 / _S2D),
}


def _n_vec():
    v = np.zeros(H, np.float64)
    for y in range(H):
        lo, hi = max(0, y - R), min(H - 1, y + R)
        v[y] = g1[lo - y + R:hi - y + R + 1].sum()
    return v


_NFULL = np.outer(_n_vec(), _n_vec()) - 1.0

W0S = (0, 96, 160)
iA = {(0, 0): 0, (0, 1): 1, (0, 2): 2, (1, 1): 3, (1, 2): 4, (2, 2): 5}
COF_PAIRS = [((1, 1), (2, 2), (1, 2), None), ((0, 2), (1, 2), (0, 1), (2, 2)),
             ((0, 1), (1, 2), (0, 2), (1, 1)), ((0, 0), (2, 2), (0, 2), None),
             ((0, 1), (0, 2), (0, 0), (1, 2)), ((0, 0), (1, 1), (0, 1), None)]
# cofactor o -> plane position in the replicated 9-plane adjugate tile
COF9_POS = {0: 0, 1: 1, 2: 2, 3: 4, 4: 5, 5: 8}


# ----------------------------------------------------------------- builder

def _build():
    nc = bacc.Bacc("TRN2", target_bir_lowering=False, debug=False,
                   enable_asserts=False, num_devices=N_CORES)
    natc = nc.dram_tensor("natc", [9, 128, NPL * 384], BF16,
                          kind="ExternalInput").ap()
    inat3 = nc.dram_tensor("inat3", [3, 3, 128, W], BF16,
                           kind="ExternalInput").ap()
    cy = nc.dram_tensor("cy", [8, 128, 64], BF16, kind="ExternalInput").ap()
    cxa = nc.dram_tensor("cxa", [32, 224], BF16, kind="ExternalInput").ap()
    cxb = nc.dram_tensor("cxb", [32, 224], BF16, kind="ExternalInput").ap()
    c2m = nc.dram_tensor("c2m", [ROWS, W], BF16, kind="ExternalInput").ap()
    bnd = {k: nc.dram_tensor(k, list(v.shape), BF16, kind="ExternalInput").ap()
           for k, v in BANDS.items()}
    tcxdr = nc.dram_tensor("tcxdr", [3, NAB, 128, W], BF16, kind="Internal").ap()
    qdr = nc.dram_tensor("qdr", [K, ROWS, W], F32, kind="ExternalOutput").ap()

    with tile.TileContext(nc) as tc, ExitStack() as top:
        cpool = top.enter_context(tc.tile_pool(name="consts", bufs=1))
        Bt = {}
        for k, v in BANDS.items():
            t = cpool.tile(list(v.shape), BF16, tag=f"band_{k}", name=f"band_{k}")
            nc.scalar.dma_start(t[:], bnd[k][:])
            Bt[k] = t
        cxat = cpool.tile([128, 224], BF16, tag="cxat", name="cxat")
        nc.scalar.dma_start(cxat[0:32, :], cxa[:])
        cxbt = cpool.tile([128, 224], BF16, tag="cxbt", name="cxbt")
        nc.scalar.dma_start(cxbt[96:128, :], cxb[:])

        with tc.tile_pool(name="pnat", bufs=3) as pnat, \
             tc.tile_pool(name="pvta", bufs=3) as pvta, \
             tc.tile_pool(name="pM", bufs=2) as pM, \
             tc.tile_pool(name="pcy", bufs=2) as pcy, \
             tc.tile_pool(name="psolve", bufs=2) as pS, \
             tc.tile_pool(name="pt4", bufs=3) as pt4, \
             tc.tile_pool(name="pg12", bufs=3) as pg12, \
             tc.tile_pool(name="pab", bufs=3) as pab, \
             tc.tile_pool(name="ptcs", bufs=2) as ptcs, \
             tc.tile_pool(name="psA", bufs=2, space="PSUM") as psA, \
             tc.tile_pool(name="psB", bufs=2, space="PSUM") as psB, \
             tc.tile_pool(name="psR", bufs=2, space="PSUM") as psR:

            vta_t = {}
            ab_t = {}

            def r1y(ch):
                # chunk 8 only feeds r1x(7)'s B32 halo: first 32 x cols used
                wdt = 32 if ch == 8 else 128
                v = pvta.tile([128, NPL * MID], BF16, tag="vta", name="vta")
                slabs = []
                for h, (p0, p1) in enumerate(((0, 13), (13, 25))):
                    sl = pnat.tile([128, 13 * 384], BF16, tag="natsub",
                                   name="natsub")
                    nc.sync.dma_start(sl[:, 0:(p1 - p0) * 384],
                                      natc[ch, :, p0 * 384:p1 * 384])
                    slabs.append(sl)
                # pack 5 consecutive 96-wide blocks per PSUM bank; one copy
                # drains 5 matmuls (flat block b = 3*pl + t covers vta
                # [96b, 96b+96))
                for ti in range(15):
                    ps = psA.tile([128, 480], F32, tag="psA", name="psA")
                    for blk in range(5):
                        fb = 5 * ti + blk
                        pl, t = divmod(fb, 3)
                        h, b0 = (0, pl * 384) if pl < 13 else (1, (pl - 13) * 384)
                        nc.tensor.matmul(ps[0:wdt, 96 * blk:96 * blk + 96],
                                         slabs[h][:, b0 + 128 * t:b0 + 128 * t + wdt],
                                         Bt["B96a"][:], start=True, stop=True,
                                         skip_group_check=True)
                    nc.scalar.copy(v[0:wdt, 480 * ti:480 * ti + 480],
                                   ps[0:wdt, :])
                vta_t[ch] = v

            def r1x(j):
                M = pM.tile([128, NPL * MID], BF16, tag="M", name="M")
                for s0 in range(0, NPL * MID, 512):
                    wdt = min(512, NPL * MID - s0)
                    ps = psB.tile([128, 512], F32, tag="psB", name="psB")
                    nc.tensor.matmul(ps[:, 0:wdt], Bt["B128"][:],
                                     vta_t[j][:, s0:s0 + wdt],
                                     start=True, stop=False)
                    nc.tensor.matmul(ps[:, 0:wdt], Bt["B32"][:],
                                     vta_t[j + 1][0:32, s0:s0 + wdt],
                                     start=False, stop=True)
                    nc.scalar.copy(M[:, s0:s0 + wdt], ps[:, 0:wdt])
                # edge-normalization fixups (bf16 constants)
                cyt = pcy.tile([128, 64], BF16, tag="cy", name="cy")
                nc.scalar.dma_start(cyt[:], cy[j])
                M3 = M[:].rearrange("p (q m) -> p q m", q=NPL)
                for (off, coff) in ((0, 0), (256, 32)):
                    cyb = cyt[:, coff:coff + 32] \
                        .rearrange("p (o f) -> p o f", o=1) \
                        .broadcast_to([128, NPL, 32])
                    nc.vector.tensor_tensor(M3[:, :, off:off + 32],
                                            M3[:, :, off:off + 32], cyb, OP.mult)
                if j in (0, 7):
                    cxt = cxat if j == 0 else cxbt
                    p0 = 0 if j == 0 else 96
                    cxB = cxt[p0:p0 + 32, :].rearrange("p (o f) -> p o f", o=1) \
                                            .broadcast_to([32, NPL, 224])
                    sl3 = M[p0:p0 + 32, :].rearrange(
                        "p (q m) -> p q m", q=NPL)[:, :, 32:256]
                    nc.vector.tensor_tensor(sl3, sl3, cxB, OP.mult)
                return M

            def solve(j, M):
                def S(p0, p1):      # plane-range slice (in cols)
                    return M[:, p0 * MID:p1 * MID]

                def r4(ap):
                    return ap.rearrange("p (k m) -> p k m", k=4)

                def bc4(ap288):
                    return ap288.rearrange("p (o m) -> p o m", o=1) \
                                .broadcast_to([128, 4, MID])

                # cov(I, p) on GpSimd (overlaps the DVE A/adjugate chain)
                covT = pS.tile([128, 12 * MID], BF16, tag="covT", name="covT")
                for c in range(3):
                    g = pt4.tile([128, 4 * MID], BF16, tag="t4", name="t4")
                    nc.gpsimd.tensor_tensor(
                        r4(g[:]), bc4(S(c, c + 1)), r4(S(3, 7)), OP.mult)
                    nc.gpsimd.tensor_tensor(
                        covT[:, c * 4 * MID:(c + 1) * 4 * MID],
                        S(7 + 4 * c, 11 + 4 * c), g[:], OP.subtract)

                # A = var(I) + eps (DVE; eps fused into the diag subtracts)
                At = pS.tile([128, 6 * MID], BF16, tag="At", name="At")
                gAs = []
                for (o0, n, c0, i0, i1) in ((0, 3, 0, 0, 3), (3, 2, 1, 1, 3),
                                            (5, 1, 2, 2, 3)):
                    g = pt4.tile([128, 4 * MID], BF16, tag="t4", name="t4")
                    q_in0 = S(c0, c0 + 1).rearrange("p (o m) -> p o m", o=1) \
                                         .broadcast_to([128, n, MID])
                    q_in1 = S(i0, i1).rearrange("p (q m) -> p q m", q=n)
                    g3 = g[:, 0:n * MID].rearrange("p (q m) -> p q m", q=n)
                    nc.vector.tensor_tensor(g3, q_in0, q_in1, OP.mult)
                    gAs.append(g)
                nc.vector.scalar_tensor_tensor(
                    At[:, 0:MID], S(19, 20), float(EPS), gAs[0][:, 0:MID],
                    OP.add, OP.subtract)
                nc.vector.tensor_tensor(At[:, MID:3 * MID], S(20, 22),
                                        gAs[0][:, MID:3 * MID], OP.subtract)
                nc.vector.scalar_tensor_tensor(
                    At[:, 3 * MID:4 * MID], S(22, 23), float(EPS),
                    gAs[1][:, 0:MID], OP.add, OP.subtract)
                nc.vector.tensor_tensor(At[:, 4 * MID:5 * MID], S(23, 24),
                                        gAs[1][:, MID:2 * MID], OP.subtract)
                nc.vector.scalar_tensor_tensor(
                    At[:, 5 * MID:6 * MID], S(24, 25), float(EPS),
                    gAs[2][:, 0:MID], OP.add, OP.subtract)

                def Ag(cc):
                    o = iA[cc]
                    return At[:, o * MID:(o + 1) * MID]

                # cofactors: all 12 products issued first (GpSimd/Scalar)
                # into one wide scratch tile, then 6 DVE subtracts into a
                # replicated 9-plane layout (cof9[3i+j]); mirrors on Scalar
                cof9 = pS.tile([128, 9 * MID], BF16, tag="cof9", name="cof9")
                cprod = pg12.tile([128, 12 * MID], BF16, tag="g12", name="g12")
                for o, (x, y, u, v) in enumerate(COF_PAIRS):
                    m1 = cprod[:, (2 * o) * MID:(2 * o + 1) * MID]
                    nc.gpsimd.tensor_tensor(m1, Ag(x), Ag(y), OP.mult)
                    m2 = cprod[:, (2 * o + 1) * MID:(2 * o + 2) * MID]
                    if v is None:
                        nc.scalar.activation(m2, Ag(u), AF.Square)
                    else:
                        nc.gpsimd.tensor_tensor(m2, Ag(u), Ag(v), OP.mult)
                for o in range(6):
                    p0 = COF9_POS[o]
                    nc.vector.tensor_tensor(
                        cof9[:, p0 * MID:(p0 + 1) * MID],
                        cprod[:, (2 * o) * MID:(2 * o + 1) * MID],
                        cprod[:, (2 * o + 1) * MID:(2 * o + 2) * MID],
                        OP.subtract)
                for dstp, srcp in ((3, 1), (6, 2), (7, 5)):
                    nc.scalar.copy(
                        cof9[:, dstp * MID:(dstp + 1) * MID],
                        cof9[:, srcp * MID:(srcp + 1) * MID])

                dtmp = pt4.tile([128, 4 * MID], BF16, tag="t4", name="t4")
                nc.vector.tensor_tensor(dtmp[:, 0:3 * MID], At[:, 0:3 * MID],
                                        cof9[:, 0:3 * MID], OP.mult)
                det = pS.tile([128, MID], F32, tag="det", name="det")
                nc.vector.tensor_tensor(det[:], dtmp[:, 0:MID],
                                        dtmp[:, MID:2 * MID], OP.add)
                nc.vector.tensor_tensor(det[:], det[:],
                                        dtmp[:, 2 * MID:3 * MID], OP.add)
                rdet = pS.tile([128, MID], F32, tag="rdet", name="rdet")
                nc.vector.reciprocal_approx_fast(rdet[:], det[:])
                rdet16 = pS.tile([128, MID], BF16, tag="rdet16", name="rdet16")
                nc.vector.tensor_copy(rdet16[:], rdet[:])

                # adjugate: scale all 9 replicated planes in place
                rb9 = rdet16[:].rearrange("p (o m) -> p o m", o=1) \
                               .broadcast_to([128, 9, MID])
                nc.vector.tensor_tensor(
                    cof9[:].rearrange("p (q m) -> p q m", q=9),
                    cof9[:].rearrange("p (q m) -> p q m", q=9), rb9, OP.mult)

                ab = pab.tile([128, NAB * MID], BF16, tag="ab", name="ab")

                # a = adj(A) @ cov: one wide multiply + 2 adds per channel
                for c in range(3):
                    mall = pg12.tile([128, 12 * MID], BF16, tag="g12",
                                     name="g12")
                    in0 = cof9[:, 3 * c * MID:(3 * c + 3) * MID] \
                        .rearrange("p (q o m) -> p q o m", q=3, o=1) \
                        .broadcast_to([128, 3, 4, MID])
                    in1 = covT[:].rearrange("p (q k m) -> p q k m", q=3, k=4)
                    o4 = mall[:].rearrange("p (q k m) -> p q k m", q=3, k=4)
                    nc.vector.tensor_tensor(o4, in0, in1, OP.mult)
                    s1 = pt4.tile([128, 4 * MID], BF16, tag="t4", name="t4")
                    nc.vector.tensor_tensor(s1[:], mall[:, 0:4 * MID],
                                            mall[:, 4 * MID:8 * MID], OP.add)
                    nc.vector.tensor_tensor(ab[:, c * 4 * MID:(c + 1) * 4 * MID],
                                            s1[:], mall[:, 8 * MID:12 * MID],
                                            OP.add)

                # b = mean_p - sum_c mean_I_c * a_c; chunks 5-6 run on
                # GpSimd (idle during the solve tail while DVE saturates)
                beng = nc.gpsimd if j in (5, 6) else nc.vector
                mb = pg12.tile([128, 12 * MID], BF16, tag="g12", name="g12")
                in0 = M[:, 0:3 * MID] \
                    .rearrange("p (q o m) -> p q o m", q=3, o=1) \
                    .broadcast_to([128, 3, 4, MID])
                in1 = ab[:, 0:12 * MID].rearrange("p (q k m) -> p q k m",
                                                  q=3, k=4)
                o4 = mb[:].rearrange("p (q k m) -> p q k m", q=3, k=4)
                beng.tensor_tensor(o4, in0, in1, OP.mult)
                s1 = pt4.tile([128, 4 * MID], BF16, tag="t4", name="t4")
                beng.tensor_tensor(s1[:], mb[:, 0:4 * MID],
                                   mb[:, 4 * MID:8 * MID], OP.add)
                s2 = pt4.tile([128, 4 * MID], BF16, tag="t4", name="t4")
                beng.tensor_tensor(s2[:], s1[:], mb[:, 8 * MID:12 * MID],
                                   OP.add)
                beng.tensor_tensor(ab[:, 12 * MID:16 * MID], S(3, 7),
                                   s2[:], OP.subtract)
                ab_t[j] = ab

            def r2x(jo):
                for w, w0 in enumerate(W0S):
                    stg = ptcs.tile([128, NAB * 128], BF16, tag="tcs",
                                    name="tcs")
                    # k-major plane order in stg/tcxdr (pg indexes k, pi
                    # indexes guide channel / b) so stage C can stream one
                    # k-lane's planes as a contiguous quarter
                    for pg in range(4):
                        ps = psR.tile([128, 512], F32, tag="psR", name="psR")
                        for pi in range(4):
                            pl = 4 * pi + pg if pi < 3 else 12 + pg
                            csl = slice(pl * MID + w0, pl * MID + w0 + 128)
                            out = ps[:, 128 * pi:128 * pi + 128]
                            # BM full width; BL/BR only touch 16 edge cols
                            nc.tensor.matmul(out, ab_t[jo][:, csl], Bt["BM"][:],
                                             start=True, stop=False,
                                             skip_group_check=True)
                            if jo > 0:
                                nc.tensor.matmul(out[:, 0:16],
                                                 ab_t[jo - 1][:, csl],
                                                 Bt["BL"][:, 0:16], start=False,
                                                 stop=(jo == 7),
                                                 skip_group_check=True)
                            if jo < 7:
                                nc.tensor.matmul(out[:, 112:128],
                                                 ab_t[jo + 1][:, csl],
                                                 Bt["BR"][:, 112:128],
                                                 start=False, stop=True,
                                                 skip_group_check=True)
                        dst = stg[:, pg * 512:pg * 512 + 512]
                        if (pg + w) % 3 != 2:
                            nc.scalar.copy(dst, ps[:])
                        else:
                            nc.vector.tensor_copy(dst, ps[:])
                    nc.sync.dma_start(
                        tcxdr[w, :, :, 128 * jo:128 * jo + 128]
                        .rearrange("q p f -> p q f"),
                        stg[:].rearrange("p (q f) -> p q f", q=NAB))

            # ---------------- pipeline ----------------
            r1y(0)
            r1y(1)
            for j in range(8):
                if j + 2 <= 8:
                    r1y(j + 2)
                M = r1x(j)
                solve(j, M)
                if j >= 1:
                    r2x(j - 1)
            r2x(7)

        # ---------------- stage C ----------------
        with tc.tile_pool(name="ptcx", bufs=2) as ptcx, \
             tc.tile_pool(name="pinat", bufs=2) as pinat, \
             tc.tile_pool(name="pq16", bufs=5) as pq16, \
             tc.tile_pool(name="pq32", bufs=2) as pq32, \
             tc.tile_pool(name="pc2", bufs=2) as pc2, \
             tc.tile_pool(name="pmt", bufs=6) as pmt, \
             tc.tile_pool(name="psC", bufs=2, space="PSUM") as psC:
            for t in range(3):
                rows = 96 if t < 2 else 64
                bg = Bt["B96b"] if t < 2 else Bt["B96c2"]
                inat_t = pinat.tile([128, 3 * W], BF16, tag="inat", name="inat")
                nc.sync.dma_start(
                    inat_t[:].rearrange("p (c f) -> p c f", c=3),
                    inat3[:, t].rearrange("c p f -> p c f"))
                c2mt = pc2.tile([96, W], BF16, tag="c2mt", name="c2mt")
                nc.scalar.dma_start(c2mt[0:rows, :], c2m[96 * t:96 * t + rows, :])
                q16s = []
                for k in range(K):
                    q16s.append(pq16.tile([96, W], BF16, tag="q16",
                                          name="q16"))
                for hh in range(2):
                    tcin = ptcx.tile([128, NAB * 512], BF16, tag="tcin",
                                     name="tcin")
                    # per-lane quarter DMAs: lane k's matmuls start as soon
                    # as its own 4 planes land
                    for kk in range(K):
                        nc.sync.dma_start(
                            tcin[:, kk * 4 * 512:(kk + 1) * 4 * 512]
                            .rearrange("p (q f) -> p q f", q=4),
                            tcxdr[t, 4 * kk:4 * kk + 4, :,
                                  512 * hh:512 * hh + 512]
                            .rearrange("q p f -> p q f"))
                    for k in range(K):
                        ps = psC.tile([96, 2048], F32, tag="psC", name="psC")
                        for ci in range(3):
                            pl = 4 * k + ci
                            nc.tensor.matmul(
                                ps[0:rows, 512 * ci:512 * ci + 512], bg[:, 0:rows],
                                tcin[:, pl * 512:(pl + 1) * 512],
                                start=True, stop=True, skip_group_check=True)
                        pl = 4 * k + 3
                        nc.tensor.matmul(ps[0:rows, 1536:2048], bg[:, 0:rows],
                                         tcin[:, pl * 512:(pl + 1) * 512],
                                         start=True, stop=True,
                                         skip_group_check=True)
                        # combine: one scalar P copy, then one wide multiply
                        # + add tree; k=3 lane runs on GpSimd
                        eng = nc.vector if k < 3 else nc.gpsimd
                        pg16 = pmt.tile([96, 2048], BF16, tag="pg", name="pg")
                        nc.scalar.copy(pg16[0:rows, :], ps[0:rows, :])
                        mall = pmt.tile([96, 1536], BF16, tag="mall",
                                        name="mall")
                        in1 = inat_t[0:rows, :] \
                            .rearrange("p (c f) -> p c f", c=3) \
                            [:, :, 512 * hh:512 * hh + 512]
                        eng.tensor_tensor(
                            mall[0:rows, :].rearrange("p (c f) -> p c f", c=3),
                            pg16[0:rows, 0:1536]
                            .rearrange("p (c f) -> p c f", c=3),
                            in1, OP.mult)
                        s1 = pmt.tile([96, 512], BF16, tag="mt", name="mt")
                        eng.tensor_tensor(s1[0:rows, :], mall[0:rows, 0:512],
                                          mall[0:rows, 512:1024], OP.add)
                        s2 = pmt.tile([96, 512], BF16, tag="mt", name="mt")
                        eng.tensor_tensor(s2[0:rows, :],
                                          mall[0:rows, 1024:1536],
                                          pg16[0:rows, 1536:2048], OP.add)
                        eng.tensor_tensor(
                            q16s[k][0:rows, 512 * hh:512 * hh + 512],
                            s1[0:rows, :], s2[0:rows, :], OP.add)
                for k in range(K):
                    q16 = q16s[k]
                    # full-width edge-normalization map (interior exactly 1)
                    eng = nc.vector if k < 3 else nc.gpsimd
                    eng.tensor_tensor(q16[0:rows, :], q16[0:rows, :],
                                      c2mt[0:rows, :], OP.mult)
                    q32 = pq32.tile([96, W], F32, tag="q32", name="q32")
                    nc.scalar.activation(q32[0:rows, :], q16[0:rows, :],
                                         AF.Copy, bias=0.5)
                    nc.sync.dma_start(qdr[k, 96 * t:96 * t + rows, :],
                                      q32[0:rows, :])

    nc.compile()
    return nc


_NC_CACHE = None


def _get_nc():
    global _NC_CACHE
    if _NC_CACHE is None:
        _NC_CACHE = _build()
    return _NC_CACHE


# ----------------------------------------------------------------- host side

def _host_prep(I, p):
    If = I.astype(np.float64) - 0.5
    pf = p.astype(np.float64) - 0.5
    band_arrs = {k: v.astype(bf16) for k, v in BANDS.items()}
    strip_cache = {}
    maps = []
    for core in range(N_CORES):
        b, i = divmod(core, 4)
        r0 = i * ROWS
        planes = [If[b, :, :, c] for c in range(C)]
        planes += [pf[b, :, :, k] for k in range(K)]
        for c in range(C):
            for k in range(K):
                planes.append(If[b, :, :, c] * pf[b, :, :, k])
        for c in range(C):
            for c2 in range(c, C):
                planes.append(If[b, :, :, c] * If[b, :, :, c2])
        planes = np.stack(planes)  # [25, H, W]

        ext = np.zeros((NPL, EXT, 1152), np.float64)
        ylo = r0 - 2 * R
        sy0, sy1 = max(0, ylo), min(H, r0 + ROWS + 2 * R)
        ext[:, sy0 - ylo:sy1 - ylo, R:R + W] = planes[:, sy0:sy1, :]

        # natc [9, 128, 25*384]: natc[ch, y, pl*384 + t*128 + x]
        natc = np.zeros((9, 128, NPL, 3, 128), np.float64)
        for t in range(3):
            blk = ext[:, 96 * t:96 * t + 128, :].transpose(1, 0, 2)
            blk = blk.reshape(128, NPL, 9, 128)
            natc[:, :, :, t, :] = blk.transpose(2, 0, 1, 3)
        natc = np.ascontiguousarray(natc.reshape(9, 128, NPL * 384)).astype(bf16)

        inat3 = np.zeros((3, 3, 128, W), np.float64)
        for c in range(C):
            for t in range(3):
                m = 96 if t < 2 else 64
                inat3[c, t, :m, :] = If[b, r0 + 96 * t:r0 + 96 * t + m, :, c]
        inat3 = inat3.astype(bf16)

        if i not in strip_cache:
            S = _S2D
            ymid0 = r0 - R
            yy = np.arange(ymid0, ymid0 + MID)
            cmid = np.zeros((MID, W))
            valid = (yy >= 0) & (yy < H)
            cmid[valid] = S / _NFULL[yy[valid]]
            cy = np.concatenate([cmid[0:32, :].T, cmid[256:288, :].T],
                                axis=1).reshape(8, 128, 64).astype(bf16)
            cxa = np.ones((32, 224), np.float64)
            cxa[0:16] = cmid[32:256, 0:16].T
            cxb = np.ones((32, 224), np.float64)
            cxb[16:32] = cmid[32:256, 1008:1024].T
            c2full = S / _NFULL[r0:r0 + ROWS]
            c2m = np.ones((ROWS, W), np.float64)
            c2m[:, 0:16] = c2full[:, 0:16]
            c2m[:, 1008:1024] = c2full[:, 1008:1024]
            if i == 0:
                c2m[0:16, 16:1008] = c2full[0:16, 16:1008]
            if i == 3:
                c2m[240:256, 16:1008] = c2full[240:256, 16:1008]
            strip_cache[i] = (cy, cxa.astype(bf16), cxb.astype(bf16),
                              c2m.astype(bf16))
        cy, cxa, cxb, c2m = strip_cache[i]

        m = dict(natc=natc, inat3=inat3, cy=cy, cxa=cxa, cxb=cxb, c2m=c2m)
        m.update(band_arrs)
        maps.append(m)
    return maps


def kernel(I, p):
    I = np.asarray(I)
    p = np.asarray(p)
    nc = _get_nc()
    in_maps = _host_prep(I, p)
    res = run_bass_kernel_spmd(nc, in_maps, core_ids=list(range(N_CORES)))
    out = np.zeros((B, H, W, K), np.float32)
    for core in range(N_CORES):
        b, i = divmod(core, 4)
        r0 = i * ROWS
        q = res.results[core]["qdr"]  # [K, ROWS, W]
        out[b, r0:r0 + ROWS] = q.transpose(1, 2, 0)
    return out


# revision 58
# speedup vs baseline: 1.0497x; 1.0497x over previous
"""Trainium2 Bass kernel for nn_BilateralLayer (guided filter, FFT-conv reference).

Fully SBUF-resident pipeline over 9 x-chunks of 128 columns. The 33x33
Gaussian (separable, zeroed center) is applied as band matmuls:
  R1y: data-stationary matmuls produce the y-filtered planes directly
       TRANSPOSED ([x, y] layout); PSUM packed 5x96 wide per bank so one
       scalar copy drains 5 matmuls.
  R1x: band-stationary matmuls (B128 + B32 halo) -> 25 moment planes.
  solve: batched 3x3 adjugate solve split across DVE + GpSimd: covT and
       cofactor products on GpSimd, At/det/adjugate/a/b on DVE with
       replicated 9-plane adjugate storage so a-compute runs as three
       wide (3456-elem) multiplies.
  R2x: ab-stationary matmuls emit the x-filtered a/b planes transposed
       back to [y, x]; BL/BR halo matmuls sliced to their 16 live output
       columns; staged through DRAM (tcxdr) for stage C.
  C:   y-band matmuls + combine q = sum_c mean_a*I + mean_b' (+0.5);
       combine split DVE (k<3) / GpSimd (k=3), final cast+bias on GpSimd.

Sharding: 8 cores = 2 batch x 4 row-bands of 256 rows (halo 2R).
Self-contained: hardcodes shapes; host-side prep in numpy.
"""
import sys

if "/opt/trn_rl_repo" not in sys.path:
    sys.path.insert(0, "/opt/trn_rl_repo")

import numpy as np
import ml_dtypes
from contextlib import ExitStack

import concourse.bass as bass
import concourse.tile as tile
from concourse import bacc, mybir
from concourse.bass_utils import run_bass_kernel_spmd

bf16 = ml_dtypes.bfloat16
F32 = mybir.dt.float32
BF16 = mybir.dt.bfloat16
F8 = mybir.dt.float8e4
OP = mybir.AluOpType
AF = mybir.ActivationFunctionType

R = 16
EPS = 0.01
B, H, W, C, K = 2, 1024, 1024, 3, 4
N_CORES = 8
ROWS = 256
EXT = ROWS + 4 * R    # 320
MID = ROWS + 2 * R    # 288
NPL = 25
NAB = 16

g1 = np.exp(-0.5 * (np.arange(-R, R + 1) ** 2) / (R / 4.0) ** 2).astype(np.float64)
_S1 = float(g1.sum())
_S2D = _S1 * _S1 - 1.0


def _band(nk, nm, shift):
    M = np.zeros((nk, nm), np.float64)
    for ki in range(nk):
        for mo in range(nm):
            d = ki - mo + shift
            if 0 <= d <= 2 * R:
                M[ki, mo] = g1[d]
    return M


BANDS = {
    "B96a": _band(128, 96, 0) / _S1,
    "B128": _band(128, 128, 0) * (_S1 / _S2D),
    "B32": _band(32, 128, 128) * (_S1 / _S2D),
    "BL": _band(128, 128, -112) / _S1,
    "BM": _band(128, 128, R) / _S1,
    "BR": _band(128, 128, 144) / _S1,
    "B96b": _band(128, 96, 0) * (_S1 / _S2D),
    "B96c2": _band(128, 64, -32) * (_S1 / _S2D),
}


def _n_vec():
    v = np.zeros(H, np.float64)
    for y in range(H):
        lo, hi = max(0, y - R), min(H - 1, y + R)
        v[y] = g1[lo - y + R:hi - y + R + 1].sum()
    return v


_NFULL = np.outer(_n_vec(), _n_vec()) - 1.0

W0S = (0, 96, 160)
iA = {(0, 0): 0, (0, 1): 1, (0, 2): 2, (1, 1): 3, (1, 2): 4, (2, 2): 5}
COF_PAIRS = [((1, 1), (2, 2), (1, 2), None), ((0, 2), (1, 2), (0, 1), (2, 2)),
             ((0, 1), (1, 2), (0, 2), (1, 1)), ((0, 0), (2, 2), (0, 2), None),
             ((0, 1), (0, 2), (0, 0), (1, 2)), ((0, 0), (1, 1), (0, 1), None)]
# cofactor o -> plane position in the replicated 9-plane adjugate tile
COF9_POS = {0: 0, 1: 1, 2: 2, 3: 4, 4: 5, 5: 8}


# ----------------------------------------------------------------- builder

def _build():
    nc = bacc.Bacc("TRN2", target_bir_lowering=False, debug=False,
                   enable_asserts=False, num_devices=N_CORES)
    natc = nc.dram_tensor("natc", [9, 128, NPL * 384], BF16,
                          kind="ExternalInput").ap()
    inat3 = nc.dram_tensor("inat3", [3, 3, 128, W], BF16,
                           kind="ExternalInput").ap()
    cy = nc.dram_tensor("cy", [8, 128, 64], BF16, kind="ExternalInput").ap()
    cxa = nc.dram_tensor("cxa", [32, 224], BF16, kind="ExternalInput").ap()
    cxb = nc.dram_tensor("cxb", [32, 224], BF16, kind="ExternalInput").ap()
    c2m = nc.dram_tensor("c2m", [ROWS, W], BF16, kind="ExternalInput").ap()
    bnd = {k: nc.dram_tensor(k, list(v.shape), BF16, kind="ExternalInput").ap()
           for k, v in BANDS.items()}
    tcxdr = nc.dram_tensor("tcxdr", [3, NAB, 128, W], BF16, kind="Internal").ap()
    qdr = nc.dram_tensor("qdr", [K, ROWS, W], F32, kind="ExternalOutput").ap()

    with tile.TileContext(nc) as tc, ExitStack() as top:
        cpool = top.enter_context(tc.tile_pool(name="consts", bufs=1))
        Bt = {}
        for k, v in BANDS.items():
            t = cpool.tile(list(v.shape), BF16, tag=f"band_{k}", name=f"band_{k}")
            nc.scalar.dma_start(t[:], bnd[k][:])
            Bt[k] = t
        cxat = cpool.tile([128, 224], BF16, tag="cxat", name="cxat")
        nc.scalar.dma_start(cxat[0:32, :], cxa[:])
        cxbt = cpool.tile([128, 224], BF16, tag="cxbt", name="cxbt")
        nc.scalar.dma_start(cxbt[96:128, :], cxb[:])

        with tc.tile_pool(name="pnat", bufs=3) as pnat, \
             tc.tile_pool(name="pvta", bufs=3) as pvta, \
             tc.tile_pool(name="pM", bufs=2) as pM, \
             tc.tile_pool(name="pcy", bufs=2) as pcy, \
             tc.tile_pool(name="psolve", bufs=2) as pS, \
             tc.tile_pool(name="pt4", bufs=3) as pt4, \
             tc.tile_pool(name="pg12", bufs=3) as pg12, \
             tc.tile_pool(name="pab", bufs=3) as pab, \
             tc.tile_pool(name="ptcs", bufs=2) as ptcs, \
             tc.tile_pool(name="psA", bufs=2, space="PSUM") as psA, \
             tc.tile_pool(name="psB", bufs=2, space="PSUM") as psB, \
             tc.tile_pool(name="psR", bufs=3, space="PSUM") as psR:

            vta_t = {}
            ab_t = {}

            def r1y(ch):
                # chunk 8 only feeds r1x(7)'s B32 halo: first 32 x cols used
                wdt = 32 if ch == 8 else 128
                v = pvta.tile([128, NPL * MID], BF16, tag="vta", name="vta")
                slabs = []
                for h, (p0, p1) in enumerate(((0, 13), (13, 25))):
                    sl = pnat.tile([128, 13 * 384], BF16, tag="natsub",
                                   name="natsub")
                    nc.sync.dma_start(sl[:, 0:(p1 - p0) * 384],
                                      natc[ch, :, p0 * 384:p1 * 384])
                    slabs.append(sl)
                # pack 5 consecutive 96-wide blocks per PSUM bank; one copy
                # drains 5 matmuls (flat block b = 3*pl + t covers vta
                # [96b, 96b+96))
                for ti in range(15):
                    ps = psA.tile([128, 480], F32, tag="psA", name="psA")
                    for blk in range(5):
                        fb = 5 * ti + blk
                        pl, t = divmod(fb, 3)
                        h, b0 = (0, pl * 384) if pl < 13 else (1, (pl - 13) * 384)
                        nc.tensor.matmul(ps[0:wdt, 96 * blk:96 * blk + 96],
                                         slabs[h][:, b0 + 128 * t:b0 + 128 * t + wdt],
                                         Bt["B96a"][:], start=True, stop=True,
                                         skip_group_check=True)
                    nc.scalar.copy(v[0:wdt, 480 * ti:480 * ti + 480],
                                   ps[0:wdt, :])
                vta_t[ch] = v

            def r1x(j):
                M = pM.tile([128, NPL * MID], BF16, tag="M", name="M")
                for s0 in range(0, NPL * MID, 512):
                    wdt = min(512, NPL * MID - s0)
                    ps = psB.tile([128, 512], F32, tag="psB", name="psB")
                    nc.tensor.matmul(ps[:, 0:wdt], Bt["B128"][:],
                                     vta_t[j][:, s0:s0 + wdt],
                                     start=True, stop=False)
                    nc.tensor.matmul(ps[:, 0:wdt], Bt["B32"][:],
                                     vta_t[j + 1][0:32, s0:s0 + wdt],
                                     start=False, stop=True)
                    nc.scalar.copy(M[:, s0:s0 + wdt], ps[:, 0:wdt])
                # edge-normalization fixups (bf16 constants)
                cyt = pcy.tile([128, 64], BF16, tag="cy", name="cy")
                nc.scalar.dma_start(cyt[:], cy[j])
                M3 = M[:].rearrange("p (q m) -> p q m", q=NPL)
                for (off, coff) in ((0, 0), (256, 32)):
                    cyb = cyt[:, coff:coff + 32] \
                        .rearrange("p (o f) -> p o f", o=1) \
                        .broadcast_to([128, NPL, 32])
                    nc.vector.tensor_tensor(M3[:, :, off:off + 32],
                                            M3[:, :, off:off + 32], cyb, OP.mult)
                if j in (0, 7):
                    cxt = cxat if j == 0 else cxbt
                    p0 = 0 if j == 0 else 96
                    cxB = cxt[p0:p0 + 32, :].rearrange("p (o f) -> p o f", o=1) \
                                            .broadcast_to([32, NPL, 224])
                    sl3 = M[p0:p0 + 32, :].rearrange(
                        "p (q m) -> p q m", q=NPL)[:, :, 32:256]
                    nc.vector.tensor_tensor(sl3, sl3, cxB, OP.mult)
                return M

            def solve(j, M):
                def S(p0, p1):      # plane-range slice (in cols)
                    return M[:, p0 * MID:p1 * MID]

                def r4(ap):
                    return ap.rearrange("p (k m) -> p k m", k=4)

                def bc4(ap288):
                    return ap288.rearrange("p (o m) -> p o m", o=1) \
                                .broadcast_to([128, 4, MID])

                # cov(I, p) on GpSimd (overlaps the DVE A/adjugate chain)
                covT = pS.tile([128, 12 * MID], BF16, tag="covT", name="covT")
                for c in range(3):
                    g = pt4.tile([128, 4 * MID], BF16, tag="t4", name="t4")
                    nc.gpsimd.tensor_tensor(
                        r4(g[:]), bc4(S(c, c + 1)), r4(S(3, 7)), OP.mult)
                    nc.gpsimd.tensor_tensor(
                        covT[:, c * 4 * MID:(c + 1) * 4 * MID],
                        S(7 + 4 * c, 11 + 4 * c), g[:], OP.subtract)

                # A = var(I) + eps (DVE; eps fused into the diag subtracts)
                At = pS.tile([128, 6 * MID], BF16, tag="At", name="At")
                gAs = []
                for (o0, n, c0, i0, i1) in ((0, 3, 0, 0, 3), (3, 2, 1, 1, 3),
                                            (5, 1, 2, 2, 3)):
                    g = pt4.tile([128, 4 * MID], BF16, tag="t4", name="t4")
                    q_in0 = S(c0, c0 + 1).rearrange("p (o m) -> p o m", o=1) \
                                         .broadcast_to([128, n, MID])
                    q_in1 = S(i0, i1).rearrange("p (q m) -> p q m", q=n)
                    g3 = g[:, 0:n * MID].rearrange("p (q m) -> p q m", q=n)
                    nc.vector.tensor_tensor(g3, q_in0, q_in1, OP.mult)
                    gAs.append(g)
                nc.vector.scalar_tensor_tensor(
                    At[:, 0:MID], S(19, 20), float(EPS), gAs[0][:, 0:MID],
                    OP.add, OP.subtract)
                nc.vector.tensor_tensor(At[:, MID:3 * MID], S(20, 22),
                                        gAs[0][:, MID:3 * MID], OP.subtract)
                nc.vector.scalar_tensor_tensor(
                    At[:, 3 * MID:4 * MID], S(22, 23), float(EPS),
                    gAs[1][:, 0:MID], OP.add, OP.subtract)
                nc.vector.tensor_tensor(At[:, 4 * MID:5 * MID], S(23, 24),
                                        gAs[1][:, MID:2 * MID], OP.subtract)
                nc.vector.scalar_tensor_tensor(
                    At[:, 5 * MID:6 * MID], S(24, 25), float(EPS),
                    gAs[2][:, 0:MID], OP.add, OP.subtract)

                def Ag(cc):
                    o = iA[cc]
                    return At[:, o * MID:(o + 1) * MID]

                # cofactors: all 12 products issued first (GpSimd/Scalar)
                # into one wide scratch tile, then 6 DVE subtracts into a
                # replicated 9-plane layout (cof9[3i+j]); mirrors on Scalar
                cof9 = pS.tile([128, 9 * MID], BF16, tag="cof9", name="cof9")
                cprod = pg12.tile([128, 12 * MID], BF16, tag="g12", name="g12")
                for o, (x, y, u, v) in enumerate(COF_PAIRS):
                    m1 = cprod[:, (2 * o) * MID:(2 * o + 1) * MID]
                    nc.gpsimd.tensor_tensor(m1, Ag(x), Ag(y), OP.mult)
                    m2 = cprod[:, (2 * o + 1) * MID:(2 * o + 2) * MID]
                    if v is None:
                        nc.scalar.activation(m2, Ag(u), AF.Square)
                    else:
                        nc.gpsimd.tensor_tensor(m2, Ag(u), Ag(v), OP.mult)
                for o in range(6):
                    p0 = COF9_POS[o]
                    nc.vector.tensor_tensor(
                        cof9[:, p0 * MID:(p0 + 1) * MID],
                        cprod[:, (2 * o) * MID:(2 * o + 1) * MID],
                        cprod[:, (2 * o + 1) * MID:(2 * o + 2) * MID],
                        OP.subtract)
                for dstp, srcp in ((3, 1), (6, 2), (7, 5)):
                    nc.scalar.copy(
                        cof9[:, dstp * MID:(dstp + 1) * MID],
                        cof9[:, srcp * MID:(srcp + 1) * MID])

                dtmp = pt4.tile([128, 4 * MID], BF16, tag="t4", name="t4")
                nc.vector.tensor_tensor(dtmp[:, 0:3 * MID], At[:, 0:3 * MID],
                                        cof9[:, 0:3 * MID], OP.mult)
                det = pS.tile([128, MID], F32, tag="det", name="det")
                nc.vector.tensor_tensor(det[:], dtmp[:, 0:MID],
                                        dtmp[:, MID:2 * MID], OP.add)
                nc.vector.tensor_tensor(det[:], det[:],
                                        dtmp[:, 2 * MID:3 * MID], OP.add)
                rdet = pS.tile([128, MID], F32, tag="rdet", name="rdet")
                nc.vector.reciprocal_approx_fast(rdet[:], det[:])
                rdet16 = pS.tile([128, MID], BF16, tag="rdet16", name="rdet16")
                nc.vector.tensor_copy(rdet16[:], rdet[:])

                # adjugate: scale all 9 replicated planes in place
                rb9 = rdet16[:].rearrange("p (o m) -> p o m", o=1) \
                               .broadcast_to([128, 9, MID])
                nc.vector.tensor_tensor(
                    cof9[:].rearrange("p (q m) -> p q m", q=9),
                    cof9[:].rearrange("p (q m) -> p q m", q=9), rb9, OP.mult)

                ab = pab.tile([128, NAB * MID], BF16, tag="ab", name="ab")

                # a = adj(A) @ cov: one wide multiply + 2 adds per channel
                for c in range(3):
                    mall = pg12.tile([128, 12 * MID], BF16, tag="g12",
                                     name="g12")
                    in0 = cof9[:, 3 * c * MID:(3 * c + 3) * MID] \
                        .rearrange("p (q o m) -> p q o m", q=3, o=1) \
                        .broadcast_to([128, 3, 4, MID])
                    in1 = covT[:].rearrange("p (q k m) -> p q k m", q=3, k=4)
                    o4 = mall[:].rearrange("p (q k m) -> p q k m", q=3, k=4)
                    nc.vector.tensor_tensor(o4, in0, in1, OP.mult)
                    s1 = pt4.tile([128, 4 * MID], BF16, tag="t4", name="t4")
                    nc.vector.tensor_tensor(s1[:], mall[:, 0:4 * MID],
                                            mall[:, 4 * MID:8 * MID], OP.add)
                    nc.vector.tensor_tensor(ab[:, c * 4 * MID:(c + 1) * 4 * MID],
                                            s1[:], mall[:, 8 * MID:12 * MID],
                                            OP.add)

                # b = mean_p - sum_c mean_I_c * a_c
                mb = pg12.tile([128, 12 * MID], BF16, tag="g12", name="g12")
                in0 = M[:, 0:3 * MID] \
                    .rearrange("p (q o m) -> p q o m", q=3, o=1) \
                    .broadcast_to([128, 3, 4, MID])
                in1 = ab[:, 0:12 * MID].rearrange("p (q k m) -> p q k m",
                                                  q=3, k=4)
                o4 = mb[:].rearrange("p (q k m) -> p q k m", q=3, k=4)
                nc.vector.tensor_tensor(o4, in0, in1, OP.mult)
                s1 = pt4.tile([128, 4 * MID], BF16, tag="t4", name="t4")
                nc.vector.tensor_tensor(s1[:], mb[:, 0:4 * MID],
                                        mb[:, 4 * MID:8 * MID], OP.add)
                s2 = pt4.tile([128, 4 * MID], BF16, tag="t4", name="t4")
                nc.vector.tensor_tensor(s2[:], s1[:], mb[:, 8 * MID:12 * MID],
                                        OP.add)
                nc.vector.tensor_tensor(ab[:, 12 * MID:16 * MID], S(3, 7),
                                        s2[:], OP.subtract)
                ab_t[j] = ab

            def r2x(jo):
                for w, w0 in enumerate(W0S):
                    stg = ptcs.tile([128, NAB * 128], BF16, tag="tcs",
                                    name="tcs")
                    # k-major plane order in stg/tcxdr (pg indexes k, pi
                    # indexes guide channel / b) so stage C can stream one
                    # k-lane's planes as a contiguous quarter
                    for pg in range(4):
                        ps = psR.tile([128, 512], F32, tag="psR", name="psR")
                        for pi in range(4):
                            pl = 4 * pi + pg if pi < 3 else 12 + pg
                            csl = slice(pl * MID + w0, pl * MID + w0 + 128)
                            out = ps[:, 128 * pi:128 * pi + 128]
                            # BM full width; BL/BR only touch 16 edge cols
                            nc.tensor.matmul(out, ab_t[jo][:, csl], Bt["BM"][:],
                                             start=True, stop=False,
                                             skip_group_check=True)
                            if jo > 0:
                                nc.tensor.matmul(out[:, 0:16],
                                                 ab_t[jo - 1][:, csl],
                                                 Bt["BL"][:, 0:16], start=False,
                                                 stop=(jo == 7),
                                                 skip_group_check=True)
                            if jo < 7:
                                nc.tensor.matmul(out[:, 112:128],
                                                 ab_t[jo + 1][:, csl],
                                                 Bt["BR"][:, 112:128],
                                                 start=False, stop=True,
                                                 skip_group_check=True)
                        dst = stg[:, pg * 512:pg * 512 + 512]
                        if (pg + w) % 3 != 2:
                            nc.scalar.copy(dst, ps[:])
                        else:
                            nc.vector.tensor_copy(dst, ps[:])
                    nc.sync.dma_start(
                        tcxdr[w, :, :, 128 * jo:128 * jo + 128]
                        .rearrange("q p f -> p q f"),
                        stg[:].rearrange("p (q f) -> p q f", q=NAB))

            # ---------------- pipeline ----------------
            r1y(0)
            r1y(1)
            for j in range(8):
                if j + 2 <= 8:
                    r1y(j + 2)
                M = r1x(j)
                solve(j, M)
                if j >= 1:
                    r2x(j - 1)
            r2x(7)

        # ---------------- stage C ----------------
        with tc.tile_pool(name="ptcx", bufs=2) as ptcx, \
             tc.tile_pool(name="pinat", bufs=2) as pinat, \
             tc.tile_pool(name="pq16", bufs=5) as pq16, \
             tc.tile_pool(name="pq32", bufs=2) as pq32, \
             tc.tile_pool(name="pc2", bufs=2) as pc2, \
             tc.tile_pool(name="pmt", bufs=6) as pmt, \
             tc.tile_pool(name="psC", bufs=2, space="PSUM") as psC:
            for t in range(3):
                rows = 96 if t < 2 else 64
                bg = Bt["B96b"] if t < 2 else Bt["B96c2"]
                inat_t = pinat.tile([128, 3 * W], BF16, tag="inat", name="inat")
                nc.sync.dma_start(
                    inat_t[:].rearrange("p (c f) -> p c f", c=3),
                    inat3[:, t].rearrange("c p f -> p c f"))
                c2mt = pc2.tile([96, W], BF16, tag="c2mt", name="c2mt")
                nc.scalar.dma_start(c2mt[0:rows, :], c2m[96 * t:96 * t + rows, :])
                q16s = []
                for k in range(K):
                    q16s.append(pq16.tile([96, W], BF16, tag="q16",
                                          name="q16"))
                for hh in range(2):
                    tcin = ptcx.tile([128, NAB * 512], BF16, tag="tcin",
                                     name="tcin")
                    # per-lane quarter DMAs: lane k's matmuls start as soon
                    # as its own 4 planes land
                    for kk in range(K):
                        nc.sync.dma_start(
                            tcin[:, kk * 4 * 512:(kk + 1) * 4 * 512]
                            .rearrange("p (q f) -> p q f", q=4),
                            tcxdr[t, 4 * kk:4 * kk + 4, :,
                                  512 * hh:512 * hh + 512]
                            .rearrange("q p f -> p q f"))
                    for k in range(K):
                        ps = psC.tile([96, 2048], F32, tag="psC", name="psC")
                        for ci in range(3):
                            pl = 4 * k + ci
                            nc.tensor.matmul(
                                ps[0:rows, 512 * ci:512 * ci + 512], bg[:, 0:rows],
                                tcin[:, pl * 512:(pl + 1) * 512],
                                start=True, stop=True, skip_group_check=True)
                        pl = 4 * k + 3
                        nc.tensor.matmul(ps[0:rows, 1536:2048], bg[:, 0:rows],
                                         tcin[:, pl * 512:(pl + 1) * 512],
                                         start=True, stop=True,
                                         skip_group_check=True)
                        # combine: one scalar P copy, then one wide multiply
                        # + add tree; k=3 lane runs on GpSimd
                        eng = nc.vector if k < 3 else nc.gpsimd
                        pg16 = pmt.tile([96, 2048], BF16, tag="pg", name="pg")
                        nc.scalar.copy(pg16[0:rows, :], ps[0:rows, :])
                        mall = pmt.tile([96, 1536], BF16, tag="mall",
                                        name="mall")
                        in1 = inat_t[0:rows, :] \
                            .rearrange("p (c f) -> p c f", c=3) \
                            [:, :, 512 * hh:512 * hh + 512]
                        eng.tensor_tensor(
                            mall[0:rows, :].rearrange("p (c f) -> p c f", c=3),
                            pg16[0:rows, 0:1536]
                            .rearrange("p (c f) -> p c f", c=3),
                            in1, OP.mult)
                        s1 = pmt.tile([96, 512], BF16, tag="mt", name="mt")
                        eng.tensor_tensor(s1[0:rows, :], mall[0:rows, 0:512],
                                          mall[0:rows, 512:1024], OP.add)
                        s2 = pmt.tile([96, 512], BF16, tag="mt", name="mt")
                        eng.tensor_tensor(s2[0:rows, :],
                                          mall[0:rows, 1024:1536],
                                          pg16[0:rows, 1536:2048], OP.add)
                        eng.tensor_tensor(
                            q16s[k][0:rows, 512 * hh:512 * hh + 512],
                            s1[0:rows, :], s2[0:rows, :], OP.add)
                for k in range(K):
                    q16 = q16s[k]
                    # full-width edge-normalization map (interior exactly 1)
                    eng = nc.vector if k < 3 else nc.gpsimd
                    eng.tensor_tensor(q16[0:rows, :], q16[0:rows, :],
                                      c2mt[0:rows, :], OP.mult)
                    q32 = pq32.tile([96, W], F32, tag="q32", name="q32")
                    nc.scalar.activation(q32[0:rows, :], q16[0:rows, :],
                                         AF.Copy, bias=0.5)
                    nc.sync.dma_start(qdr[k, 96 * t:96 * t + rows, :],
                                      q32[0:rows, :])

    nc.compile()
    return nc


_NC_CACHE = None


def _get_nc():
    global _NC_CACHE
    if _NC_CACHE is None:
        _NC_CACHE = _build()
    return _NC_CACHE


# ----------------------------------------------------------------- host side

def _host_prep(I, p):
    If = I.astype(np.float64) - 0.5
    pf = p.astype(np.float64) - 0.5
    band_arrs = {k: v.astype(bf16) for k, v in BANDS.items()}
    strip_cache = {}
    maps = []
    for core in range(N_CORES):
        b, i = divmod(core, 4)
        r0 = i * ROWS
        planes = [If[b, :, :, c] for c in range(C)]
        planes += [pf[b, :, :, k] for k in range(K)]
        for c in range(C):
            for k in range(K):
                planes.append(If[b, :, :, c] * pf[b, :, :, k])
        for c in range(C):
            for c2 in range(c, C):
                planes.append(If[b, :, :, c] * If[b, :, :, c2])
        planes = np.stack(planes)  # [25, H, W]

        ext = np.zeros((NPL, EXT, 1152), np.float64)
        ylo = r0 - 2 * R
        sy0, sy1 = max(0, ylo), min(H, r0 + ROWS + 2 * R)
        ext[:, sy0 - ylo:sy1 - ylo, R:R + W] = planes[:, sy0:sy1, :]

        # natc [9, 128, 25*384]: natc[ch, y, pl*384 + t*128 + x]
        natc = np.zeros((9, 128, NPL, 3, 128), np.float64)
        for t in range(3):
            blk = ext[:, 96 * t:96 * t + 128, :].transpose(1, 0, 2)
            blk = blk.reshape(128, NPL, 9, 128)
            natc[:, :, :, t, :] = blk.transpose(2, 0, 1, 3)
        natc = np.ascontiguousarray(natc.reshape(9, 128, NPL * 384)).astype(bf16)

        inat3 = np.zeros((3, 3, 128, W), np.float64)
        for c in range(C):
            for t in range(3):
                m = 96 if t < 2 else 64
                inat3[c, t, :m, :] = If[b, r0 + 96 * t:r0 + 96 * t + m, :, c]
        inat3 = inat3.astype(bf16)

        if i not in strip_cache:
            S = _S2D
            ymid0 = r0 - R
            yy = np.arange(ymid0, ymid0 + MID)
            cmid = np.zeros((MID, W))
            valid = (yy >= 0) & (yy < H)
            cmid[valid] = S / _NFULL[yy[valid]]
            cy = np.concatenate([cmid[0:32, :].T, cmid[256:288, :].T],
                                axis=1).reshape(8, 128, 64).astype(bf16)
            cxa = np.ones((32, 224), np.float64)
            cxa[0:16] = cmid[32:256, 0:16].T
            cxb = np.ones((32, 224), np.float64)
            cxb[16:32] = cmid[32:256, 1008:1024].T
            c2full = S / _NFULL[r0:r0 + ROWS]
            c2m = np.ones((ROWS, W), np.float64)
            c2m[:, 0:16] = c2full[:, 0:16]
            c2m[:, 1008:1024] = c2full[:, 1008:1024]
            if i == 0:
                c2m[0:16, 16:1008] = c2full[0:16, 16:1008]
            if i == 3:
                c2m[240:256, 16:1008] = c2full[240:256, 16:1008]
            strip_cache[i] = (cy, cxa.astype(bf16), cxb.astype(bf16),
                              c2m.astype(bf16))
        cy, cxa, cxb, c2m = strip_cache[i]

        m = dict(natc=natc, inat3=inat3, cy=cy, cxa=cxa, cxb=cxb, c2m=c2m)
        m.update(band_arrs)
        maps.append(m)
    return maps


def kernel(I, p):
    I = np.asarray(I)
    p = np.asarray(p)
    nc = _get_nc()
    in_maps = _host_prep(I, p)
    res = run_bass_kernel_spmd(nc, in_maps, core_ids=list(range(N_CORES)))
    out = np.zeros((B, H, W, K), np.float32)
    for core in range(N_CORES):
        b, i = divmod(core, 4)
        r0 = i * ROWS
        q = res.results[core]["qdr"]  # [K, ROWS, W]
        out[b, r0:r0 + ROWS] = q.transpose(1, 2, 0)
    return out
